# revision 75
# baseline (speedup 1.0000x reference)
"""Multi-head self-attention Trainium2 kernel (8 NeuronCores).

Problem: x[2,2048,1024] -> qkv proj (w_qkv[1024,3072]) -> 16-head attention
(head_dim 64) -> out proj (w_out[1024,1024]).

Sharding: core c handles batch b=c//4 and head-group g=c%4 (4 heads each).
Each core computes Q/K/V for its 4 heads (tensor-parallel slice of w_qkv),
runs attention for those heads, and computes a partial out-projection
(rows g*256:(g+1)*256 of w_out). The host sums the 4 partials per batch and
adds b_out plus the constant b_v @ w_out term (softmax weights sum to 1, so
the V-bias contributes a constant row that never needs to touch the device).

Everything on device is fp16 (inputs, Q/K/V, exp tiles, ctx, weights) with
fp32 PSUM accumulation; rel err lands ~1e-3, well under the 2e-2 gate, and
fp16 halves DMA traffic and removes the f32r small-N matmul penalty.

Layouts (per core):
  XT  [128, 8, 2048]   x^T (d-major), d = dk*128 + p
  QT/KT [128, 2, 2048]  channel-major Q^T/K^T; head h at partitions
                        (h%2)*64..+64 of chunk h//2
  V4  [128, 16, 4, 66] sequence-major V per k-chunk/head; col 64 = ones
                       (gives softmax denominators for free in attn@V),
                       col 65 = zero padding for even matmul width
  et  [128, 1024]      exp(scores/8) tiles, k on partitions, q on free
  CTXQ [128, 16, 4, 64] q-major context (q on partitions) accumulated from
                       attn@V with M=128 (full PE column use, half the PE
                       rows of the old 65-wide c-major form)
  CTXT [128, 2, 2048]  c-major ctx for the out projection, produced by
                       XBAR dma transposes (idle DMA engines, no PE/DVE)

The schedule keeps the Activation engine (softmax exp, the ~133us critical
resource) streaming continuously: a minimal preamble (Q for the first
q-half, K for the first four k-chunks) starts the exp stream early, and all
remaining QKV work (V per-chunk, K/Q chunks) runs as deadline-paced "jobs"
in the tensor-engine spare time inside the attention passes.
"""

import os
from collections import deque
from contextlib import ExitStack

import numpy as np

import concourse.bacc as bacc
import concourse.mybir as mybir
import concourse.tile as tile
from concourse.bass_utils import run_bass_kernel_spmd

P = 128
B, S, D, H, HD = 2, 2048, 1024, 16, 64
HPC = 4          # heads per core
C = HPC * HD     # 256 channels per core
DK = D // P      # 8 contraction chunks
CT = C // P      # 2 channel chunks
SC = S // P      # 16 sequence chunks of 128
QW = 512         # matmul q-slice width
PW = 1024        # attention pass q-half width / st tile width
VW = HD + 2      # V4 cols per head: 64 data + ones + zero pad (even N)
F32 = mybir.dt.float32
F16 = mybir.dt.float16
AF = mybir.ActivationFunctionType

N_CORES = 8
CORES_PER_BATCH = 4


class _Jobs:
    """Deadline-paced emission of deferred matmul work into PE spare time.

    Each job is a list of mm-emitters plus a finalizer (the PSUM->SBUF
    copy). At each pass sub-iteration, overdue work is emitted
    unconditionally and remaining budget (in matmul rows) is filled from
    the queue front, so QKV jobs never starve the score-matmul stream.
    """

    def __init__(self):
        self.q = deque()

    def add(self, mms, fin, due, rows, deadline=None):
        self.q.append([list(mms), fin, due, rows, deadline])

    def remaining_rows(self):
        return sum(len(mms) * rows for mms, fin, due, rows, dl in self.q)

    def force(self, it):
        # hard deadlines: fully emit any job whose consumer runs this
        # sub-iteration (program order is the only correctness guarantee)
        while self.q and self.q[0][4] is not None and self.q[0][4] <= it:
            mms, fin, due, rows, dl = self.q.popleft()
            for m in mms:
                m()
            if fin is not None:
                fin()

    def emit(self, it, budget_rows):
        # FIFO in deadline order, capped per sub-iteration so job bursts
        # never starve the score-matmul -> exp stream
        while self.q and budget_rows > 0:
            mms, fin, due, rows, dl = self.q[0]
            while mms and budget_rows > 0:
                mms.pop(0)()
                budget_rows -= rows
            if mms:
                return
            if fin is not None:
                fin()
            self.q.popleft()

    def drain(self):
        self.emit(1 << 30, 1 << 30)


def _build():
    nc = bacc.Bacc("TRN2", target_bir_lowering=False, debug=False)
    xt = nc.dram_tensor("xt", (D, S), F16, kind="ExternalInput")
    # weights arrive host-packed p-major (partition-contiguous rows) so
    # every weight DMA moves >=2KB descriptors at full rate
    wq = nc.dram_tensor("wq", (P, CT, DK, P), F16, kind="ExternalInput")
    wk = nc.dram_tensor("wk", (P, CT, DK, P), F16, kind="ExternalInput")
    wv = nc.dram_tensor("wv", (P, CT, DK, P), F16, kind="ExternalInput")
    wo = nc.dram_tensor("wo", (P, CT, D), F16, kind="ExternalInput")
    bqk = nc.dram_tensor("bqk", (2, C), F32, kind="ExternalInput")
    out = nc.dram_tensor("out", (D, S), F16, kind="ExternalOutput")
    dbg = bool(int(os.environ.get("BASS_KERNEL_DEBUG", "0")))
    if dbg:
        d_qt = nc.dram_tensor("d_qt", (P, CT * S), F16, kind="ExternalOutput")
        d_kt = nc.dram_tensor("d_kt", (P, CT * S), F16, kind="ExternalOutput")
        d_v4 = nc.dram_tensor("d_v4", (P, SC * HPC * VW), F16,
                              kind="ExternalOutput")
        d_cq = nc.dram_tensor("d_cq", (P, SC * HPC * HD), F16,
                              kind="ExternalOutput")
        d_ct = nc.dram_tensor("d_ct", (P, 2 * 8 * CT * P), F16,
                              kind="ExternalOutput")

    xt_r = xt.rearrange("(dk p) s -> p dk s", p=P)

    with tile.TileContext(nc) as tc, ExitStack() as ctx:
        pers = ctx.enter_context(tc.tile_pool(name="pers", bufs=1))
        XT = pers.tile([P, DK, S], F16)
        WQ = pers.tile([P, DK, C], F16)
        WK = pers.tile([P, DK, C], F16)
        WV = pers.tile([P, DK, C], F16)
        WO = pers.tile([P, CT, D], F16)
        QT = pers.tile([P, CT, S], F16)
        KT = pers.tile([P, CT, S], F16)
        V4 = pers.tile([P, SC, HPC, VW], F16)
        CTXQ = pers.tile([P, SC, HPC, HD], F16)
        # c-major ctx, c-chunks interleaved per q-chunk so one XBAR dma
        # transpose per q-half produces it: CTXT[p, qh, qc, cc, q] holds
        # ctx^T[cc*128+p, qh*1024+qc*128+q]
        CTXT = pers.tile([P, 2, 8, CT, P], F16)
        BQK = pers.tile([P, 2, CT], F32)

        etp = ctx.enter_context(tc.tile_pool(name="et", bufs=4))
        osbp = ctx.enter_context(tc.tile_pool(name="osb", bufs=16))
        recp = ctx.enter_context(tc.tile_pool(name="rec", bufs=2))
        stp = ctx.enter_context(tc.tile_pool(name="st", bufs=2, space="PSUM"))
        pop = ctx.enter_context(tc.tile_pool(name="po", bufs=1, space="PSUM"))
        jbp = ctx.enter_context(tc.tile_pool(name="jb", bufs=1, space="PSUM"))

        # V4 denominator-ones and pad columns (written once; V copies fill
        # the data columns)
        nc.gpsimd.memset(V4[:, :, :, HD], 1.0)
        nc.gpsimd.memset(V4[:, :, :, HD + 1], 0.0)
        WRM = pers.tile([1, 2], F16)
        nc.gpsimd.memset(WRM, 0.0)

        # ---- DMA: weights first as whole-tensor copies (HWDGE issue slots
        # are the scarce resource at ~650ns each), then s-progressive x^T
        # chunks so attention can start on the first q-half / k-chunks
        # while the rest streams in ----
        # wq/wk ct0 halves lead (the preamble's only weights, 364ns each
        # thanks to p-major packing), interleaved with the s[0:512] x^T
        # pairs the preamble consumes; everything else streams behind
        nc.sync.dma_start(WQ[:, :, 0:P], wq[:, 0].rearrange("p dk c -> p dk c"))
        for dk in range(0, 4, 2):
            nc.sync.dma_start(XT[:, dk:dk + 2, 0:QW], xt_r[:, dk:dk + 2, 0:QW])
        nc.sync.dma_start(WK[:, :, 0:P], wk[:, 0].rearrange("p dk c -> p dk c"))
        for dk in range(4, DK, 2):
            nc.sync.dma_start(XT[:, dk:dk + 2, 0:QW], xt_r[:, dk:dk + 2, 0:QW])
        nc.sync.dma_start(BQK, bqk.rearrange("qk (ct p) -> p qk ct", p=P))
        nc.sync.dma_start(WV[:, :, 0:P], wv[:, 0].rearrange("p dk c -> p dk c"))
        nc.sync.dma_start(WV[:, :, P:C], wv[:, 1].rearrange("p dk c -> p dk c"))
        for dk in range(0, DK, 2):
            nc.sync.dma_start(XT[:, dk:dk + 2, QW:PW], xt_r[:, dk:dk + 2, QW:PW])
        nc.sync.dma_start(WQ[:, :, P:C], wq[:, 1].rearrange("p dk c -> p dk c"))
        nc.sync.dma_start(WK[:, :, P:C], wk[:, 1].rearrange("p dk c -> p dk c"))
        for dk in range(0, DK, 2):
            nc.sync.dma_start(XT[:, dk:dk + 2, PW:S], xt_r[:, dk:dk + 2, PW:S])
        nc.sync.dma_start(WO, wo[:, :, :])

        # warm-up: a negligible matmul as early as possible starts the PE
        # p-state ramp (~11us to full clock) during the DMA preamble
        wps = jbp.tile([1, 2], F32, tag="jb", name="wps")
        nc.tensor.matmul(wps, lhsT=WRM[:, 0:1], rhs=WRM, start=True, stop=True)

        # ---- preamble: Q ct0 q[0:512] plus K ct0 k-chunks 0-1, chunk-paced
        # against the x^T DMA stream (one Q + two K matmuls fit in a chunk
        # interval), then Q q[512:1024]. Pass 0 starts half-width so the
        # exp stream fires as soon as the first Q half is copied. ----
        qa = stp.tile([P, QW], F32, tag="st", name="preQa")
        kp = jbp.tile([P, 2 * P], F32, tag="jb", name="preK")
        for dk in range(DK):
            nc.tensor.matmul(
                qa, lhsT=WQ[:, dk, 0:P], rhs=XT[:, dk, 0:QW],
                start=(dk == 0), stop=(dk == DK - 1))
            nc.tensor.matmul(
                kp, lhsT=WK[:, dk, 0:P], rhs=XT[:, dk, 0:2 * P],
                start=(dk == 0), stop=(dk == DK - 1))
        nc.vector.tensor_scalar_add(QT[:, 0, 0:QW], qa, BQK[:, 0, 0:1])
        nc.vector.tensor_scalar_add(KT[:, 0, 0:2 * P], kp, BQK[:, 1, 0:1])

        def q_second_half():
            qb = stp.tile([P, QW], F32, tag="st", name="preQb")

            def mm(dk):
                return lambda: nc.tensor.matmul(
                    qb, lhsT=WQ[:, dk, 0:P], rhs=XT[:, dk, QW:PW],
                    start=(dk == 0), stop=(dk == DK - 1))

            def fin():
                nc.vector.tensor_scalar_add(
                    QT[:, 0, QW:PW], qb, BQK[:, 0, 0:1])
            return [mm(dk) for dk in range(DK)], fin

        # ---- deferred QKV work as jobs ----
        def v_job(st, vhp):
            """V for k-chunk st, head-pair vhp only (the pass that consumes
            a head-pair also computes its V, halving pass-0's job load)."""
            jb = jbp.tile([P, P], F32, tag="jb", name=f"vj{vhp}_{st}")

            def mm(dk):
                return lambda: nc.tensor.matmul(
                    jb, lhsT=XT[:, dk, st * P:(st + 1) * P],
                    rhs=WV[:, dk, vhp * P:(vhp + 1) * P],
                    start=(dk == 0), stop=(dk == DK - 1))

            def fin():
                nc.vector.tensor_copy(
                    V4[:, st, 2 * vhp:2 * vhp + 2, 0:HD],
                    jb.rearrange("p (h d) -> p h d", d=HD))
            return [mm(dk) for dk in range(DK)], fin

        def k_chunk_job(ct_i, kc):
            jb = jbp.tile([P, P], F32, tag="jb", name=f"kj{ct_i}_{kc}")

            def mm(dk):
                return lambda: nc.tensor.matmul(
                    jb, lhsT=WK[:, dk, ct_i * P:(ct_i + 1) * P],
                    rhs=XT[:, dk, kc * P:(kc + 1) * P],
                    start=(dk == 0), stop=(dk == DK - 1))

            def fin():
                nc.vector.tensor_scalar_add(
                    KT[:, ct_i, kc * P:(kc + 1) * P], jb,
                    BQK[:, 1, ct_i:ct_i + 1])
            return [mm(dk) for dk in range(DK)], fin

        def q_job(ct_i, qc):
            jb = jbp.tile([P, QW], F32, tag="jb", name=f"qj{ct_i}_{qc}")

            def mm(dk):
                return lambda: nc.tensor.matmul(
                    jb, lhsT=WQ[:, dk, ct_i * P:(ct_i + 1) * P],
                    rhs=XT[:, dk, qc * QW:(qc + 1) * QW],
                    start=(dk == 0), stop=(dk == DK - 1))

            def fin():
                nc.vector.tensor_scalar_add(
                    QT[:, ct_i, qc * QW:(qc + 1) * QW], jb,
                    BQK[:, 0, ct_i:ct_i + 1])
            return [mm(dk) for dk in range(DK)], fin

        # per-pass job queues. Sub-iteration index it = kc*2 + hh (0..31).
        # Jobs are interleaved in deadline order so V chunks and K chunks
        # arrive just in time for the kc loop that consumes them.
        pass_jobs = [_Jobs() for _ in range(4)]
        p0 = []
        # pass 0 ((hp0,qh0)): Q q[512:1024] paced across the half-width
        # units, V st1.. JIT, K ct0 kc4..15 JIT, Q ct0 qh1
        # (V st0 is emitted inline right after the third pass-0 unit)
        # v_ready[hp]: k-chunks whose V copy has been emitted (attn@V for a
        # chunk must not be emitted before its V job — program order is the
        # only ordering guarantee)
        v_ready = [set(), set()]

        def v_job_r(st_i, vhp):
            mms, fin = v_job(st_i, vhp)

            def fin_r():
                fin()
                v_ready[vhp].add(st_i)
            return mms, fin_r

        # pass-0 unit index of the first score matmul that reads K chunk kc
        def p0_k_deadline(kc):
            return 2 * kc if kc < 4 else 16 + 2 * (kc - 4)

        for kc in (2, 3):
            mms, fin = k_chunk_job(0, kc)
            p0.append((kc - 2, mms, fin, P, p0_k_deadline(kc) - 1))
        mms, fin = q_second_half()
        p0.append((1, mms, fin, QW, 7))
        for st_i in range(1, SC):
            mms, fin = v_job_r(st_i, 0)
            p0.append((max(1, 2 * st_i + 2), mms, fin, P, None))
        for kc in range(4, SC):
            mms, fin = k_chunk_job(0, kc)
            p0.append((max(1, 2 * kc), mms, fin, P, p0_k_deadline(kc) - 1))
        for qc in (2, 3):
            mms, fin = q_job(0, qc)
            p0.append((14 + 4 * qc, mms, fin, QW, None))
        for due, mms, fin, rows, dl in sorted(p0, key=lambda t: t[0]):
            pass_jobs[0].add(mms, fin, due=due, rows=rows, deadline=dl)
        # pass 1 ((hp0,qh1)): K ct1 all chunks, Q ct1 qh0
        for kc in range(SC):
            mms, fin = k_chunk_job(1, kc)
            pass_jobs[1].add(mms, fin, due=2 * kc, rows=P)
        for qc in (0, 1):
            mms, fin = q_job(1, qc)
            pass_jobs[1].add(mms, fin, due=20 + 5 * qc, rows=QW)
        # pass 2 ((hp1,qh0)): V for heads 2-3 JIT, Q ct1 qh1
        for st_i in range(SC):
            mms, fin = v_job_r(st_i, 1)
            pass_jobs[2].add(mms, fin, due=max(0, 2 * st_i - 4), rows=P)
        for qc in (2, 3):
            mms, fin = q_job(1, qc)
            pass_jobs[2].add(mms, fin, due=8 * qc, rows=QW)

        # ---- out-projection helpers ----
        def oproj_unit(nn, qh, store_eng, slot=1):
            """out^T[nn*128:+128, qh*1024:+1024] = WO^T-chunk @ CTXT: four
            bank-aligned matmuls into one 2-bank psum tile, one wide store,
            one DMA (fewer stores/sems than per-512 units)."""
            pool, tag = ((jbp, "jb"), (stp, "st"), (pop, "po"))[slot]
            ps = pool.tile([P, PW], F32, tag=tag, name=f"op{nn}_{qh}")
            for half in range(2):
                for cc in range(CT):
                    nc.tensor.matmul(
                        ps[:, half * QW:(half + 1) * QW],
                        lhsT=WO[:, cc, nn * P:(nn + 1) * P],
                        rhs=CTXT[:, qh, half * 4:half * 4 + 4, cc, :],
                        start=(cc == 0), stop=(cc == CT - 1))
            osb = osbp.tile([P, PW], F16, tag="osb", name=f"osb{nn}_{qh}")
            # halves drain on both store engines concurrently
            nc.scalar.copy(osb[:, 0:QW], ps[:, 0:QW])
            nc.vector.tensor_copy(osb[:, QW:PW], ps[:, QW:PW])
            nc.sync.dma_start(
                out[nn * P:(nn + 1) * P, qh * PW:(qh + 1) * PW], osb)

        # out-proj for qh0 runs inside pass 3 ((hp1,qh1)) as per-512 halves
        # through the single jb bank (the score ring is busy), paced
        osb_q0 = {}

        def oproj_half(nn, half, slot=0):
            pool, tag = ((jbp, "jb"), (stp, "st"), (pop, "po"))[slot]
            ps = pool.tile([P, QW], F32, tag=tag, name=f"oph{nn}_{half}")
            for cc in range(CT):
                nc.tensor.matmul(
                    ps, lhsT=WO[:, cc, nn * P:(nn + 1) * P],
                    rhs=CTXT[:, 0, half * 4:half * 4 + 4, cc, :],
                    start=(cc == 0), stop=(cc == CT - 1))
            if nn not in osb_q0:
                osb_q0[nn] = osbp.tile([P, PW], F16, tag="osb",
                                       name=f"osbq0_{nn}")
            nc.vector.tensor_copy(
                osb_q0[nn][:, half * QW:(half + 1) * QW], ps)
            if half == 1:
                nc.sync.dma_start(
                    out[nn * P:(nn + 1) * P, 0:PW], osb_q0.pop(nn))

        op_halves = deque((nn, h) for nn in range(DK) for h in (0, 1))

        # ---- 4 attention passes of (head-pair, q-half) ----
        # Pass 0's first eight kc are split into half-width (512-q) units
        # so exps start right after the first Q half lands; V st0 and the
        # second Q half are slotted between those early units.
        for pi, (hp, qh) in enumerate(((0, 0), (0, 1), (1, 0), (1, 1))):
            po = pop.tile([P, 8, 2, VW], F32, tag="po", name=f"po{pi}")
            # 16 (m, hh) accumulation groups share po's banks; a start=True
            # would pending-zero a whole 2KB bank and wipe its siblings, so
            # zero the tile once and accumulate throughout
            nc.vector.memset(po, 0.0)
            q0 = qh * PW

            def attn_v(pend):
                et, kc, hh, mlo, mhi = pend
                for m in range(mlo, mhi):
                    nc.tensor.matmul(
                        po[:, m, hh, :],
                        lhsT=et[:, (m - mlo) * P:(m - mlo + 1) * P],
                        rhs=V4[:, kc, 2 * hp + hh, :],
                        start=False, stop=(kc == SC - 1),
                        skip_group_check=True)

            if pi == 0:
                units = ([(kc, hh, 0, 1) for kc in range(4) for hh in (0, 1)]
                         + [(kc, hh, 1, 2) for kc in range(4) for hh in (0, 1)]
                         + [(kc, hh, 0, 2) for kc in range(4, SC)
                            for hh in (0, 1)])
            else:
                units = [(kc, hh, 0, 2) for kc in range(SC) for hh in (0, 1)]

            pends = []
            for it, (kc, hh, jlo, jhi) in enumerate(units):
                pass_jobs[pi].force(it)
                w = (jhi - jlo) * QW
                st = stp.tile([P, w], F32, tag="st", name=f"st{pi}")
                for j in range(jlo, jhi):
                    nc.tensor.matmul(
                        st[:, (j - jlo) * QW:(j - jlo + 1) * QW],
                        lhsT=KT[hh * 64:(hh + 1) * 64, hp,
                                kc * P:(kc + 1) * P],
                        rhs=QT[hh * 64:(hh + 1) * 64, hp,
                               q0 + j * QW:q0 + (j + 1) * QW],
                        start=True, stop=True)
                while len(pends) >= 2 and pends[0][1] in v_ready[hp]:
                    attn_v(pends.pop(0))
                et = etp.tile([P, w], F16, tag="et", name="et")
                nc.scalar.activation(et, st, AF.Exp, scale=0.125)
                pends.append((et, kc, hh, jlo * 4, jhi * 4))
                if pi == 0 and it == 0:
                    # V chunk 0, needed by the first attn@V two units later
                    mms, fin = v_job_r(0, 0)
                    for m in mms:
                        m()
                    fin()
                else:
                    # pace the queue evenly over the pass's remaining units
                    rem = pass_jobs[pi].remaining_rows()
                    pace = (rem * 13) // (10 * max(1, len(units) - it))
                    pass_jobs[pi].emit(it, max(512, pace))
                if pi == 3 and op_halves and it % 2 == 1 and it >= 13:
                    oproj_half(*op_halves.popleft())
            pass_jobs[pi].drain()
            for pend in pends:
                attn_v(pend)

            # normalize: ctxq[q, h, hd] = po[q, h, hd] / po[q, h, 64];
            # split per q-half-of-pass on the last pass so the transpose
            # and the tail out-projection start as early as possible
            halves = ((0, 4), (4, 8)) if pi == 3 else ((0, 8),)
            for lo, hi in halves:
                rec = recp.tile([P, hi - lo, 2], F32, tag="rec", name="rec")
                nc.vector.reciprocal(rec, po[:, lo:hi, :, HD])
                nc.vector.tensor_mul(
                    CTXQ[:, qh * 8 + lo:qh * 8 + hi, 2 * hp:2 * hp + 2, :],
                    po[:, lo:hi, :, 0:HD],
                    rec.unsqueeze(3).to_broadcast((P, hi - lo, 2, HD)))
                if hp == 1:
                    # both head-pairs of this q-half done: XBAR transpose
                    # on the (idle) DMA engines makes the c-major form.
                    # The last pass issues from the ACT queue (idle after
                    # its final exp), skipping the SP-queue backlog on the
                    # critical tail path.
                    eng = nc.scalar if pi == 3 else nc.sync
                    eng.dma_start_transpose(
                        CTXT[:, qh, lo:hi].rearrange(
                            "p qc cc q -> p (qc cc) q"),
                        CTXQ[:, qh * 8 + lo:qh * 8 + hi])

        # leftover qh0 work bridges the transpose latency (it only needs
        # the long-ready qh0 ctx), keeping the PE hot for the qh1 tail;
        # everything streams as wide units over the three 2-bank psum
        # slots (st x2 + the freed po) with both store engines in play
        while op_halves:
            oproj_half(*op_halves.popleft())
        for n_tail, nn in enumerate(range(DK)):
            oproj_unit(nn, 1, "both", slot=1 + (n_tail % 3 == 2))

        if dbg:
            nc.sync.dma_start(d_qt[:, :], QT.rearrange("p a b -> p (a b)"))
            nc.sync.dma_start(d_kt[:, :], KT.rearrange("p a b -> p (a b)"))
            nc.sync.dma_start(d_v4[:, :], V4.rearrange("p a b c -> p (a b c)"))
            nc.sync.dma_start(d_cq[:, :], CTXQ.rearrange("p a b c -> p (a b c)"))
            nc.sync.dma_start(d_ct[:, :], CTXT.rearrange("p a b c d -> p (a b c d)"))

    nc.compile()
    return nc


_NC = None


def _pack_w(w):
    # [D, C] -> p-major [p, ct, dk, c0] so DMA rows are >=2KB
    return np.ascontiguousarray(
        w.reshape(DK, P, CT, P).transpose(1, 2, 0, 3)).astype(np.float16)


def core_inputs(core, x, w_qkv, b_qkv, w_out):
    b_i, g = divmod(core, CORES_PER_BATCH)
    cs = slice(g * HPC * HD, (g + 1) * HPC * HD)  # this core's channels
    return {
        "xt": np.ascontiguousarray(x[b_i].T).astype(np.float16),
        "wq": _pack_w(w_qkv[:, 0 * D:1 * D][:, cs]),
        "wk": _pack_w(w_qkv[:, 1 * D:2 * D][:, cs]),
        "wv": _pack_w(w_qkv[:, 2 * D:3 * D][:, cs]),
        "wo": np.ascontiguousarray(
            w_out[cs, :].reshape(CT, P, D).transpose(1, 0, 2)
        ).astype(np.float16),
        "bqk": np.ascontiguousarray(
            np.stack([b_qkv[0 * D:1 * D][cs], b_qkv[1 * D:2 * D][cs]])),
    }


def kernel(x, w_qkv, b_qkv, w_out, b_out):
    global _NC
    x = np.asarray(x, dtype=np.float32)
    w_qkv = np.asarray(w_qkv, dtype=np.float32)
    b_qkv = np.asarray(b_qkv, dtype=np.float32)
    w_out = np.asarray(w_out, dtype=np.float32)
    b_out = np.asarray(b_out, dtype=np.float32)

    if _NC is None:
        _NC = _build()

    in_maps = [core_inputs(core, x, w_qkv, b_qkv, w_out)
               for core in range(N_CORES)]

    trace = bool(int(os.environ.get("BASS_KERNEL_TRACE", "0")))
    res = run_bass_kernel_spmd(
        _NC, in_maps, core_ids=list(range(N_CORES)), trace=trace,
    )
    if trace and res.exec_time_ns is not None:
        print(f"HW exec time: {res.exec_time_ns} ns")
        if res.instructions_and_trace is not None:
            print(f"trace: {res.instructions_and_trace[1]}")

    # b_out and the V-bias term (softmax weights sum to 1, so x@w_v bias
    # b_v contributes the constant row b_v @ w_out) are host-folded.
    bias_row = b_out + b_qkv[2 * D:3 * D] @ w_out

    outs = [r["out"] for r in res.results]
    full = np.empty((B, S, D), dtype=np.float32)
    for b_i in range(B):
        acc = np.sum(
            np.stack(outs[b_i * CORES_PER_BATCH:(b_i + 1) * CORES_PER_BATCH]),
            axis=0, dtype=np.float32,
        ).T
        full[b_i] = acc + bias_row[None, :]
    return full


# revision 79
# speedup vs baseline: 1.0029x; 1.0029x over previous
"""Multi-head self-attention Trainium2 kernel (8 NeuronCores).

Problem: x[2,2048,1024] -> qkv proj (w_qkv[1024,3072]) -> 16-head attention
(head_dim 64) -> out proj (w_out[1024,1024]).

Sharding: core c handles batch b=c//4 and head-group g=c%4 (4 heads each).
Each core computes Q/K/V for its 4 heads (tensor-parallel slice of w_qkv),
runs attention for those heads, and computes a partial out-projection
(rows g*256:(g+1)*256 of w_out). The host sums the 4 partials per batch and
adds b_out plus the constant b_v @ w_out term (softmax weights sum to 1, so
the V-bias contributes a constant row that never needs to touch the device).

Everything on device is fp16 (inputs, Q/K/V, exp tiles, ctx, weights) with
fp32 PSUM accumulation; rel err lands ~1e-3, well under the 2e-2 gate, and
fp16 halves DMA traffic and removes the f32r small-N matmul penalty.

Layouts (per core):
  XT  [128, 8, 2048]   x^T (d-major), d = dk*128 + p
  QT/KT [128, 2, 2048]  channel-major Q^T/K^T; head h at partitions
                        (h%2)*64..+64 of chunk h//2
  V4  [128, 16, 4, 66] sequence-major V per k-chunk/head; col 64 = ones
                       (gives softmax denominators for free in attn@V),
                       col 65 = zero padding for even matmul width
  et  [128, 1024]      exp(scores/8) tiles, k on partitions, q on free
  CTXQ [128, 16, 4, 64] q-major context (q on partitions) accumulated from
                       attn@V with M=128 (full PE column use, half the PE
                       rows of the old 65-wide c-major form)
  CTXT [128, 2, 2048]  c-major ctx for the out projection, produced by
                       XBAR dma transposes (idle DMA engines, no PE/DVE)

The schedule keeps the Activation engine (softmax exp, the ~133us critical
resource) streaming continuously: a minimal preamble (Q for the first
q-half, K for the first four k-chunks) starts the exp stream early, and all
remaining QKV work (V per-chunk, K/Q chunks) runs as deadline-paced "jobs"
in the tensor-engine spare time inside the attention passes.
"""

import os
from collections import deque
from contextlib import ExitStack

import numpy as np

import concourse.bacc as bacc
import concourse.mybir as mybir
import concourse.tile as tile
from concourse.bass_utils import run_bass_kernel_spmd

P = 128
B, S, D, H, HD = 2, 2048, 1024, 16, 64
HPC = 4          # heads per core
C = HPC * HD     # 256 channels per core
DK = D // P      # 8 contraction chunks
CT = C // P      # 2 channel chunks
SC = S // P      # 16 sequence chunks of 128
QW = 512         # matmul q-slice width
PW = 1024        # attention pass q-half width / st tile width
VW = HD + 2      # V4 cols per head: 64 data + ones + zero pad (even N)
F32 = mybir.dt.float32
F16 = mybir.dt.float16
AF = mybir.ActivationFunctionType

N_CORES = 8
CORES_PER_BATCH = 4


class _Jobs:
    """Deadline-paced emission of deferred matmul work into PE spare time.

    Each job is a list of mm-emitters plus a finalizer (the PSUM->SBUF
    copy). At each pass sub-iteration, overdue work is emitted
    unconditionally and remaining budget (in matmul rows) is filled from
    the queue front, so QKV jobs never starve the score-matmul stream.
    """

    def __init__(self):
        self.q = deque()

    def add(self, mms, fin, due, rows, deadline=None):
        self.q.append([list(mms), fin, due, rows, deadline])

    def remaining_rows(self):
        return sum(len(mms) * rows for mms, fin, due, rows, dl in self.q)

    def force(self, it):
        # hard deadlines: fully emit any job whose consumer runs this
        # sub-iteration (program order is the only correctness guarantee)
        while self.q and self.q[0][4] is not None and self.q[0][4] <= it:
            mms, fin, due, rows, dl = self.q.popleft()
            for m in mms:
                m()
            if fin is not None:
                fin()

    def emit(self, it, budget_rows):
        # FIFO in deadline order, capped per sub-iteration so job bursts
        # never starve the score-matmul -> exp stream
        while self.q and budget_rows > 0:
            mms, fin, due, rows, dl = self.q[0]
            while mms and budget_rows > 0:
                mms.pop(0)()
                budget_rows -= rows
            if mms:
                return
            if fin is not None:
                fin()
            self.q.popleft()

    def drain(self):
        self.emit(1 << 30, 1 << 30)


def _build():
    nc = bacc.Bacc("TRN2", target_bir_lowering=False, debug=False)
    xt = nc.dram_tensor("xt", (D, S), F16, kind="ExternalInput")
    # weights arrive host-packed p-major (partition-contiguous rows) so
    # every weight DMA moves >=2KB descriptors at full rate
    wq = nc.dram_tensor("wq", (P, CT, DK, P), F16, kind="ExternalInput")
    wk = nc.dram_tensor("wk", (P, CT, DK, P), F16, kind="ExternalInput")
    wv = nc.dram_tensor("wv", (P, CT, DK, P), F16, kind="ExternalInput")
    wo = nc.dram_tensor("wo", (P, CT, D), F16, kind="ExternalInput")
    bqk = nc.dram_tensor("bqk", (2, C), F32, kind="ExternalInput")
    out = nc.dram_tensor("out", (D, S), F16, kind="ExternalOutput")
    dbg = bool(int(os.environ.get("BASS_KERNEL_DEBUG", "0")))
    if dbg:
        d_qt = nc.dram_tensor("d_qt", (P, CT * S), F16, kind="ExternalOutput")
        d_kt = nc.dram_tensor("d_kt", (P, CT * S), F16, kind="ExternalOutput")
        d_v4 = nc.dram_tensor("d_v4", (P, SC * HPC * VW), F16,
                              kind="ExternalOutput")
        d_cq = nc.dram_tensor("d_cq", (P, SC * HPC * HD), F16,
                              kind="ExternalOutput")
        d_ct = nc.dram_tensor("d_ct", (P, 2 * 8 * CT * P), F16,
                              kind="ExternalOutput")

    xt_r = xt.rearrange("(dk p) s -> p dk s", p=P)

    with tile.TileContext(nc) as tc, ExitStack() as ctx:
        pers = ctx.enter_context(tc.tile_pool(name="pers", bufs=1))
        XT = pers.tile([P, DK, S], F16)
        WQ = pers.tile([P, DK, C], F16)
        WK = pers.tile([P, DK, C], F16)
        WV = pers.tile([P, DK, C], F16)
        WO = pers.tile([P, CT, D], F16)
        QT = pers.tile([P, CT, S], F16)
        KT = pers.tile([P, CT, S], F16)
        V4 = pers.tile([P, SC, HPC, VW], F16)
        CTXQ = pers.tile([P, SC, HPC, HD], F16)
        # c-major ctx, c-chunks interleaved per q-chunk so one XBAR dma
        # transpose per q-half produces it: CTXT[p, qh, qc, cc, q] holds
        # ctx^T[cc*128+p, qh*1024+qc*128+q]
        CTXT = pers.tile([P, 2, 8, CT, P], F16)
        BQK = pers.tile([P, 2, CT], F32)

        etp = ctx.enter_context(tc.tile_pool(name="et", bufs=6))
        osbp = ctx.enter_context(tc.tile_pool(name="osb", bufs=16))
        recp = ctx.enter_context(tc.tile_pool(name="rec", bufs=4))
        stp = ctx.enter_context(tc.tile_pool(name="st", bufs=2, space="PSUM"))
        pop = ctx.enter_context(tc.tile_pool(name="po", bufs=1, space="PSUM"))
        jbp = ctx.enter_context(tc.tile_pool(name="jb", bufs=1, space="PSUM"))

        # V4 denominator-ones and pad columns (written once; V copies fill
        # the data columns)
        nc.gpsimd.memset(V4[:, :, :, HD], 1.0)
        nc.gpsimd.memset(V4[:, :, :, HD + 1], 0.0)
        WRM = pers.tile([1, 2], F16)
        nc.gpsimd.memset(WRM, 0.0)

        # ---- DMA: weights first as whole-tensor copies (HWDGE issue slots
        # are the scarce resource at ~650ns each), then s-progressive x^T
        # chunks so attention can start on the first q-half / k-chunks
        # while the rest streams in ----
        # wq/wk ct0 halves lead (the preamble's only weights, 364ns each
        # thanks to p-major packing), interleaved with the s[0:512] x^T
        # pairs the preamble consumes; everything else streams behind
        nc.sync.dma_start(WQ[:, :, 0:P], wq[:, 0].rearrange("p dk c -> p dk c"))
        for dk in range(0, 4, 2):
            nc.sync.dma_start(XT[:, dk:dk + 2, 0:QW], xt_r[:, dk:dk + 2, 0:QW])
        nc.sync.dma_start(WK[:, :, 0:P], wk[:, 0].rearrange("p dk c -> p dk c"))
        for dk in range(4, DK, 2):
            nc.sync.dma_start(XT[:, dk:dk + 2, 0:QW], xt_r[:, dk:dk + 2, 0:QW])
        nc.sync.dma_start(BQK, bqk.rearrange("qk (ct p) -> p qk ct", p=P))
        nc.sync.dma_start(WV[:, :, 0:P], wv[:, 0].rearrange("p dk c -> p dk c"))
        nc.sync.dma_start(WV[:, :, P:C], wv[:, 1].rearrange("p dk c -> p dk c"))
        for dk in range(0, DK, 2):
            nc.sync.dma_start(XT[:, dk:dk + 2, QW:PW], xt_r[:, dk:dk + 2, QW:PW])
        nc.sync.dma_start(WQ[:, :, P:C], wq[:, 1].rearrange("p dk c -> p dk c"))
        nc.sync.dma_start(WK[:, :, P:C], wk[:, 1].rearrange("p dk c -> p dk c"))
        for dk in range(0, DK, 2):
            nc.sync.dma_start(XT[:, dk:dk + 2, PW:S], xt_r[:, dk:dk + 2, PW:S])
        nc.sync.dma_start(WO, wo[:, :, :])

        # warm-up: a negligible matmul as early as possible starts the PE
        # p-state ramp (~11us to full clock) during the DMA preamble
        wps = jbp.tile([1, 2], F32, tag="jb", name="wps")
        nc.tensor.matmul(wps, lhsT=WRM[:, 0:1], rhs=WRM, start=True, stop=True)

        # ---- preamble: Q ct0 q[0:512] plus K ct0 k-chunks 0-1, chunk-paced
        # against the x^T DMA stream (one Q + two K matmuls fit in a chunk
        # interval), then Q q[512:1024]. Pass 0 starts half-width so the
        # exp stream fires as soon as the first Q half is copied. ----
        qa = stp.tile([P, QW], F32, tag="st", name="preQa")
        kp = jbp.tile([P, 2 * P], F32, tag="jb", name="preK")
        for dk in range(DK):
            nc.tensor.matmul(
                qa, lhsT=WQ[:, dk, 0:P], rhs=XT[:, dk, 0:QW],
                start=(dk == 0), stop=(dk == DK - 1))
            nc.tensor.matmul(
                kp, lhsT=WK[:, dk, 0:P], rhs=XT[:, dk, 0:2 * P],
                start=(dk == 0), stop=(dk == DK - 1))
        nc.vector.tensor_scalar_add(QT[:, 0, 0:QW], qa, BQK[:, 0, 0:1])
        nc.vector.tensor_scalar_add(KT[:, 0, 0:2 * P], kp, BQK[:, 1, 0:1])

        def q_second_half():
            qb = stp.tile([P, QW], F32, tag="st", name="preQb")

            def mm(dk):
                return lambda: nc.tensor.matmul(
                    qb, lhsT=WQ[:, dk, 0:P], rhs=XT[:, dk, QW:PW],
                    start=(dk == 0), stop=(dk == DK - 1))

            def fin():
                nc.vector.tensor_scalar_add(
                    QT[:, 0, QW:PW], qb, BQK[:, 0, 0:1])
            return [mm(dk) for dk in range(DK)], fin

        # ---- deferred QKV work as jobs ----
        def v_job(st, vhp):
            """V for k-chunk st, head-pair vhp only (the pass that consumes
            a head-pair also computes its V, halving pass-0's job load)."""
            jb = jbp.tile([P, P], F32, tag="jb", name=f"vj{vhp}_{st}")

            def mm(dk):
                return lambda: nc.tensor.matmul(
                    jb, lhsT=XT[:, dk, st * P:(st + 1) * P],
                    rhs=WV[:, dk, vhp * P:(vhp + 1) * P],
                    start=(dk == 0), stop=(dk == DK - 1))

            def fin():
                nc.vector.tensor_copy(
                    V4[:, st, 2 * vhp:2 * vhp + 2, 0:HD],
                    jb.rearrange("p (h d) -> p h d", d=HD))
            return [mm(dk) for dk in range(DK)], fin

        def k_chunk_job(ct_i, kc):
            jb = jbp.tile([P, P], F32, tag="jb", name=f"kj{ct_i}_{kc}")

            def mm(dk):
                return lambda: nc.tensor.matmul(
                    jb, lhsT=WK[:, dk, ct_i * P:(ct_i + 1) * P],
                    rhs=XT[:, dk, kc * P:(kc + 1) * P],
                    start=(dk == 0), stop=(dk == DK - 1))

            def fin():
                nc.vector.tensor_scalar_add(
                    KT[:, ct_i, kc * P:(kc + 1) * P], jb,
                    BQK[:, 1, ct_i:ct_i + 1])
            return [mm(dk) for dk in range(DK)], fin

        def q_job(ct_i, qc):
            jb = jbp.tile([P, QW], F32, tag="jb", name=f"qj{ct_i}_{qc}")

            def mm(dk):
                return lambda: nc.tensor.matmul(
                    jb, lhsT=WQ[:, dk, ct_i * P:(ct_i + 1) * P],
                    rhs=XT[:, dk, qc * QW:(qc + 1) * QW],
                    start=(dk == 0), stop=(dk == DK - 1))

            def fin():
                nc.vector.tensor_scalar_add(
                    QT[:, ct_i, qc * QW:(qc + 1) * QW], jb,
                    BQK[:, 0, ct_i:ct_i + 1])
            return [mm(dk) for dk in range(DK)], fin

        # per-pass job queues. Sub-iteration index it = kc*2 + hh (0..31).
        # Jobs are interleaved in deadline order so V chunks and K chunks
        # arrive just in time for the kc loop that consumes them.
        pass_jobs = [_Jobs() for _ in range(4)]
        p0 = []
        # pass 0 ((hp0,qh0)): Q q[512:1024] paced across the half-width
        # units, V st1.. JIT, K ct0 kc4..15 JIT, Q ct0 qh1
        # (V st0 is emitted inline right after the third pass-0 unit)
        # v_ready[hp]: k-chunks whose V copy has been emitted (attn@V for a
        # chunk must not be emitted before its V job — program order is the
        # only ordering guarantee)
        v_ready = [set(), set()]

        def v_job_r(st_i, vhp):
            mms, fin = v_job(st_i, vhp)

            def fin_r():
                fin()
                v_ready[vhp].add(st_i)
            return mms, fin_r

        # pass-0 unit index of the first score matmul that reads K chunk kc
        def p0_k_deadline(kc):
            return 2 * kc if kc < 4 else 16 + 2 * (kc - 4)

        for kc in (2, 3):
            mms, fin = k_chunk_job(0, kc)
            p0.append((kc - 2, mms, fin, P, p0_k_deadline(kc) - 1))
        mms, fin = q_second_half()
        p0.append((1, mms, fin, QW, 7))
        for st_i in range(1, SC):
            mms, fin = v_job_r(st_i, 0)
            p0.append((max(1, 2 * st_i + 2), mms, fin, P, None))
        for kc in range(4, SC):
            mms, fin = k_chunk_job(0, kc)
            p0.append((max(1, 2 * kc), mms, fin, P, p0_k_deadline(kc) - 1))
        for qc in (2, 3):
            mms, fin = q_job(0, qc)
            p0.append((14 + 4 * qc, mms, fin, QW, None))
        for due, mms, fin, rows, dl in sorted(p0, key=lambda t: t[0]):
            pass_jobs[0].add(mms, fin, due=due, rows=rows, deadline=dl)
        # pass 1 ((hp0,qh1)): K ct1 all chunks, Q ct1 qh0
        for kc in range(SC):
            mms, fin = k_chunk_job(1, kc)
            pass_jobs[1].add(mms, fin, due=2 * kc, rows=P)
        for qc in (0, 1):
            mms, fin = q_job(1, qc)
            pass_jobs[1].add(mms, fin, due=20 + 5 * qc, rows=QW)
        # pass 2 ((hp1,qh0)): V for heads 2-3 JIT, Q ct1 qh1
        for st_i in range(SC):
            mms, fin = v_job_r(st_i, 1)
            pass_jobs[2].add(mms, fin, due=max(0, 2 * st_i - 4), rows=P)
        for qc in (2, 3):
            mms, fin = q_job(1, qc)
            pass_jobs[2].add(mms, fin, due=8 * qc, rows=QW)

        # ---- out-projection helpers ----
        def oproj_unit(nn, qh, store_eng, slot=1):
            """out^T[nn*128:+128, qh*1024:+1024] = WO^T-chunk @ CTXT: four
            bank-aligned matmuls into one 2-bank psum tile, one wide store,
            one DMA (fewer stores/sems than per-512 units)."""
            pool, tag = ((jbp, "jb"), (stp, "st"), (pop, "po"))[slot]
            ps = pool.tile([P, PW], F32, tag=tag, name=f"op{nn}_{qh}")
            for half in range(2):
                for cc in range(CT):
                    nc.tensor.matmul(
                        ps[:, half * QW:(half + 1) * QW],
                        lhsT=WO[:, cc, nn * P:(nn + 1) * P],
                        rhs=CTXT[:, qh, half * 4:half * 4 + 4, cc, :],
                        start=(cc == 0), stop=(cc == CT - 1))
            osb = osbp.tile([P, PW], F16, tag="osb", name=f"osb{nn}_{qh}")
            # halves drain on both store engines concurrently
            nc.scalar.copy(osb[:, 0:QW], ps[:, 0:QW])
            nc.vector.tensor_copy(osb[:, QW:PW], ps[:, QW:PW])
            nc.sync.dma_start(
                out[nn * P:(nn + 1) * P, qh * PW:(qh + 1) * PW], osb)

        # out-proj for qh0 runs inside pass 3 ((hp1,qh1)) as per-512 halves
        # through the single jb bank (the score ring is busy), paced
        osb_q0 = {}

        def oproj_half(nn, half, slot=0):
            pool, tag = ((jbp, "jb"), (stp, "st"), (pop, "po"))[slot]
            ps = pool.tile([P, QW], F32, tag=tag, name=f"oph{nn}_{half}")
            for cc in range(CT):
                nc.tensor.matmul(
                    ps, lhsT=WO[:, cc, nn * P:(nn + 1) * P],
                    rhs=CTXT[:, 0, half * 4:half * 4 + 4, cc, :],
                    start=(cc == 0), stop=(cc == CT - 1))
            if nn not in osb_q0:
                osb_q0[nn] = osbp.tile([P, PW], F16, tag="osb",
                                       name=f"osbq0_{nn}")
            nc.vector.tensor_copy(
                osb_q0[nn][:, half * QW:(half + 1) * QW], ps)
            if half == 1:
                nc.sync.dma_start(
                    out[nn * P:(nn + 1) * P, 0:PW], osb_q0.pop(nn))

        op_halves = deque((nn, h) for nn in range(DK) for h in (0, 1))

        # ---- 4 attention passes of (head-pair, q-half) ----
        # Pass 0's first eight kc are split into half-width (512-q) units
        # so exps start right after the first Q half lands; V st0 and the
        # second Q half are slotted between those early units.
        for pi, (hp, qh) in enumerate(((0, 0), (0, 1), (1, 0), (1, 1))):
            po = pop.tile([P, 8, 2, VW], F32, tag="po", name=f"po{pi}")
            # 16 (m, hh) accumulation groups share po's banks; a start=True
            # would pending-zero a whole 2KB bank and wipe its siblings, so
            # zero the tile once and accumulate throughout
            nc.vector.memset(po, 0.0)
            q0 = qh * PW

            def attn_v(pend):
                et, kc, hh, mlo, mhi = pend
                for m in range(mlo, mhi):
                    nc.tensor.matmul(
                        po[:, m, hh, :],
                        lhsT=et[:, (m - mlo) * P:(m - mlo + 1) * P],
                        rhs=V4[:, kc, 2 * hp + hh, :],
                        start=False, stop=(kc == SC - 1),
                        skip_group_check=True)

            if pi == 0:
                units = ([(kc, hh, 0, 1) for kc in range(4) for hh in (0, 1)]
                         + [(kc, hh, 1, 2) for kc in range(4) for hh in (0, 1)]
                         + [(kc, hh, 0, 2) for kc in range(4, SC)
                            for hh in (0, 1)])
            else:
                units = [(kc, hh, 0, 2) for kc in range(SC) for hh in (0, 1)]

            pends = []
            for it, (kc, hh, jlo, jhi) in enumerate(units):
                pass_jobs[pi].force(it)
                w = (jhi - jlo) * QW
                st = stp.tile([P, w], F32, tag="st", name=f"st{pi}")
                for j in range(jlo, jhi):
                    nc.tensor.matmul(
                        st[:, (j - jlo) * QW:(j - jlo + 1) * QW],
                        lhsT=KT[hh * 64:(hh + 1) * 64, hp,
                                kc * P:(kc + 1) * P],
                        rhs=QT[hh * 64:(hh + 1) * 64, hp,
                               q0 + j * QW:q0 + (j + 1) * QW],
                        start=True, stop=True)
                while len(pends) >= 2 and pends[0][1] in v_ready[hp]:
                    attn_v(pends.pop(0))
                et = etp.tile([P, w], F16, tag="et", name="et")
                nc.scalar.activation(et, st, AF.Exp, scale=0.125)
                pends.append((et, kc, hh, jlo * 4, jhi * 4))
                if pi == 0 and it == 0:
                    # V chunk 0, needed by the first attn@V two units later
                    mms, fin = v_job_r(0, 0)
                    for m in mms:
                        m()
                    fin()
                else:
                    # pace the queue evenly over the pass's remaining units
                    rem = pass_jobs[pi].remaining_rows()
                    pace = (rem * 13) // (10 * max(1, len(units) - it))
                    pass_jobs[pi].emit(it, max(512, pace))
                if pi == 3 and op_halves and it % 2 == 1 and it >= 13:
                    oproj_half(*op_halves.popleft())
            pass_jobs[pi].drain()
            for pend in pends:
                attn_v(pend)

            # normalize: ctxq[q, h, hd] = po[q, h, hd] / po[q, h, 64];
            # split per q-half-of-pass on the last pass so the transpose
            # and the tail out-projection start as early as possible
            halves = ((0, 4), (4, 8)) if pi == 3 else ((0, 8),)
            for lo, hi in halves:
                rec = recp.tile([P, hi - lo, 2], F32, tag="rec", name="rec")
                nc.vector.reciprocal(rec, po[:, lo:hi, :, HD])
                nc.vector.tensor_mul(
                    CTXQ[:, qh * 8 + lo:qh * 8 + hi, 2 * hp:2 * hp + 2, :],
                    po[:, lo:hi, :, 0:HD],
                    rec.unsqueeze(3).to_broadcast((P, hi - lo, 2, HD)))
                if hp == 1:
                    # both head-pairs of this q-half done: XBAR transpose
                    # on the (idle) DMA engines makes the c-major form.
                    # The last pass issues from the ACT queue (idle after
                    # its final exp), skipping the SP-queue backlog on the
                    # critical tail path.
                    eng = nc.scalar if pi == 3 else nc.sync
                    eng.dma_start_transpose(
                        CTXT[:, qh, lo:hi].rearrange(
                            "p qc cc q -> p (qc cc) q"),
                        CTXQ[:, qh * 8 + lo:qh * 8 + hi])
                if pi == 3 and lo == 0:
                    # a few bridge rows between the two norm halves keep
                    # the PE hot and their stores ahead of norm-B on DVE
                    for _ in range(3):
                        if op_halves:
                            oproj_half(*op_halves.popleft())

        # leftover qh0 work bridges the transpose latency (it only needs
        # the long-ready qh0 ctx), keeping the PE hot for the qh1 tail;
        # everything streams as wide units over the three 2-bank psum
        # slots (st x2 + the freed po) with both store engines in play
        while op_halves:
            oproj_half(*op_halves.popleft())
        for n_tail, nn in enumerate(range(DK)):
            oproj_unit(nn, 1, "both", slot=1 + (n_tail % 3 == 2))

        if dbg:
            nc.sync.dma_start(d_qt[:, :], QT.rearrange("p a b -> p (a b)"))
            nc.sync.dma_start(d_kt[:, :], KT.rearrange("p a b -> p (a b)"))
            nc.sync.dma_start(d_v4[:, :], V4.rearrange("p a b c -> p (a b c)"))
            nc.sync.dma_start(d_cq[:, :], CTXQ.rearrange("p a b c -> p (a b c)"))
            nc.sync.dma_start(d_ct[:, :], CTXT.rearrange("p a b c d -> p (a b c d)"))

    nc.compile()
    return nc


_NC = None


def _pack_w(w):
    # [D, C] -> p-major [p, ct, dk, c0] so DMA rows are >=2KB
    return np.ascontiguousarray(
        w.reshape(DK, P, CT, P).transpose(1, 2, 0, 3)).astype(np.float16)


def core_inputs(core, x, w_qkv, b_qkv, w_out):
    b_i, g = divmod(core, CORES_PER_BATCH)
    cs = slice(g * HPC * HD, (g + 1) * HPC * HD)  # this core's channels
    return {
        "xt": np.ascontiguousarray(x[b_i].T).astype(np.float16),
        "wq": _pack_w(w_qkv[:, 0 * D:1 * D][:, cs]),
        "wk": _pack_w(w_qkv[:, 1 * D:2 * D][:, cs]),
        "wv": _pack_w(w_qkv[:, 2 * D:3 * D][:, cs]),
        "wo": np.ascontiguousarray(
            w_out[cs, :].reshape(CT, P, D).transpose(1, 0, 2)
        ).astype(np.float16),
        "bqk": np.ascontiguousarray(
            np.stack([b_qkv[0 * D:1 * D][cs], b_qkv[1 * D:2 * D][cs]])),
    }


def kernel(x, w_qkv, b_qkv, w_out, b_out):
    global _NC
    x = np.asarray(x, dtype=np.float32)
    w_qkv = np.asarray(w_qkv, dtype=np.float32)
    b_qkv = np.asarray(b_qkv, dtype=np.float32)
    w_out = np.asarray(w_out, dtype=np.float32)
    b_out = np.asarray(b_out, dtype=np.float32)

    if _NC is None:
        _NC = _build()

    in_maps = [core_inputs(core, x, w_qkv, b_qkv, w_out)
               for core in range(N_CORES)]

    trace = bool(int(os.environ.get("BASS_KERNEL_TRACE", "0")))
    res = run_bass_kernel_spmd(
        _NC, in_maps, core_ids=list(range(N_CORES)), trace=trace,
    )
    if trace and res.exec_time_ns is not None:
        print(f"HW exec time: {res.exec_time_ns} ns")
        if res.instructions_and_trace is not None:
            print(f"trace: {res.instructions_and_trace[1]}")

    # b_out and the V-bias term (softmax weights sum to 1, so x@w_v bias
    # b_v contributes the constant row b_v @ w_out) are host-folded.
    bias_row = b_out + b_qkv[2 * D:3 * D] @ w_out

    outs = [r["out"] for r in res.results]
    full = np.empty((B, S, D), dtype=np.float32)
    for b_i in range(B):
        acc = np.sum(
            np.stack(outs[b_i * CORES_PER_BATCH:(b_i + 1) * CORES_PER_BATCH]),
            axis=0, dtype=np.float32,
        ).T
        full[b_i] = acc + bias_row[None, :]
    return full


# revision 82
# speedup vs baseline: 1.0100x; 1.0070x over previous
"""Multi-head self-attention Trainium2 kernel (8 NeuronCores).

Problem: x[2,2048,1024] -> qkv proj (w_qkv[1024,3072]) -> 16-head attention
(head_dim 64) -> out proj (w_out[1024,1024]).

Sharding: core c handles batch b=c//4 and head-group g=c%4 (4 heads each).
Each core computes Q/K/V for its 4 heads (tensor-parallel slice of w_qkv),
runs attention for those heads, and computes a partial out-projection
(rows g*256:(g+1)*256 of w_out). The host sums the 4 partials per batch and
adds b_out plus the constant b_v @ w_out term (softmax weights sum to 1, so
the V-bias contributes a constant row that never needs to touch the device).

Everything on device is fp16 (inputs, Q/K/V, exp tiles, ctx, weights) with
fp32 PSUM accumulation; rel err lands ~1e-3, well under the 2e-2 gate, and
fp16 halves DMA traffic and removes the f32r small-N matmul penalty.

Layouts (per core):
  XT  [128, 8, 2048]   x^T (d-major), d = dk*128 + p
  QT/KT [128, 2, 2048]  channel-major Q^T/K^T; head h at partitions
                        (h%2)*64..+64 of chunk h//2
  V4  [128, 16, 4, 66] sequence-major V per k-chunk/head; col 64 = ones
                       (gives softmax denominators for free in attn@V),
                       col 65 = zero padding for even matmul width
  et  [128, 1024]      exp(scores/8) tiles, k on partitions, q on free
  CTXQ [128, 16, 4, 64] q-major context (q on partitions) accumulated from
                       attn@V with M=128 (full PE column use, half the PE
                       rows of the old 65-wide c-major form)
  CTXT [128, 2, 2048]  c-major ctx for the out projection, produced by
                       XBAR dma transposes (idle DMA engines, no PE/DVE)

The schedule keeps the Activation engine (softmax exp, the ~133us critical
resource) streaming continuously: a minimal preamble (Q for the first
q-half, K for the first four k-chunks) starts the exp stream early, and all
remaining QKV work (V per-chunk, K/Q chunks) runs as deadline-paced "jobs"
in the tensor-engine spare time inside the attention passes.
"""

import os
from collections import deque
from contextlib import ExitStack

import numpy as np

import concourse.bacc as bacc
import concourse.mybir as mybir
import concourse.tile as tile
from concourse.bass_utils import run_bass_kernel_spmd

P = 128
B, S, D, H, HD = 2, 2048, 1024, 16, 64
HPC = 4          # heads per core
C = HPC * HD     # 256 channels per core
DK = D // P      # 8 contraction chunks
CT = C // P      # 2 channel chunks
SC = S // P      # 16 sequence chunks of 128
QW = 512         # matmul q-slice width
PW = 1024        # attention pass q-half width / st tile width
VW = HD + 2      # V4 cols per head: 64 data + ones + zero pad (even N)
F32 = mybir.dt.float32
F16 = mybir.dt.float16
AF = mybir.ActivationFunctionType

N_CORES = 8
CORES_PER_BATCH = 4


class _Jobs:
    """Deadline-paced emission of deferred matmul work into PE spare time.

    Each job is a list of mm-emitters plus a finalizer (the PSUM->SBUF
    copy). At each pass sub-iteration, overdue work is emitted
    unconditionally and remaining budget (in matmul rows) is filled from
    the queue front, so QKV jobs never starve the score-matmul stream.
    """

    def __init__(self):
        self.q = deque()

    def add(self, mms, fin, due, rows, deadline=None):
        self.q.append([list(mms), fin, due, rows, deadline])

    def remaining_rows(self):
        return sum(len(mms) * rows for mms, fin, due, rows, dl in self.q)

    def force(self, it):
        # hard deadlines: fully emit any job whose consumer runs this
        # sub-iteration (program order is the only correctness guarantee)
        while self.q and self.q[0][4] is not None and self.q[0][4] <= it:
            mms, fin, due, rows, dl = self.q.popleft()
            for m in mms:
                m()
            if fin is not None:
                fin()

    def emit(self, it, budget_rows):
        # FIFO in deadline order, capped per sub-iteration so job bursts
        # never starve the score-matmul -> exp stream
        while self.q and budget_rows > 0:
            mms, fin, due, rows, dl = self.q[0]
            while mms and budget_rows > 0:
                mms.pop(0)()
                budget_rows -= rows
            if mms:
                return
            if fin is not None:
                fin()
            self.q.popleft()

    def drain(self):
        self.emit(1 << 30, 1 << 30)


def _build():
    nc = bacc.Bacc("TRN2", target_bir_lowering=False, debug=False)
    xt = nc.dram_tensor("xt", (D, S), F16, kind="ExternalInput")
    # weights arrive host-packed p-major (partition-contiguous rows) so
    # every weight DMA moves >=2KB descriptors at full rate
    wq = nc.dram_tensor("wq", (P, CT, DK, P), F16, kind="ExternalInput")
    wk = nc.dram_tensor("wk", (P, CT, DK, P), F16, kind="ExternalInput")
    wv = nc.dram_tensor("wv", (P, CT, DK, P), F16, kind="ExternalInput")
    wo = nc.dram_tensor("wo", (P, CT, D), F16, kind="ExternalInput")
    bqk = nc.dram_tensor("bqk", (2, C), F32, kind="ExternalInput")
    out = nc.dram_tensor("out", (D, S), F16, kind="ExternalOutput")
    dbg = bool(int(os.environ.get("BASS_KERNEL_DEBUG", "0")))
    if dbg:
        d_qt = nc.dram_tensor("d_qt", (P, CT * S), F16, kind="ExternalOutput")
        d_kt = nc.dram_tensor("d_kt", (P, CT * S), F16, kind="ExternalOutput")
        d_v4 = nc.dram_tensor("d_v4", (P, SC * HPC * VW), F16,
                              kind="ExternalOutput")
        d_cq = nc.dram_tensor("d_cq", (P, SC * HPC * HD), F16,
                              kind="ExternalOutput")
        d_ct = nc.dram_tensor("d_ct", (P, 2 * 8 * CT * P), F16,
                              kind="ExternalOutput")

    xt_r = xt.rearrange("(dk p) s -> p dk s", p=P)

    with tile.TileContext(nc) as tc, ExitStack() as ctx:
        pers = ctx.enter_context(tc.tile_pool(name="pers", bufs=1))
        XT = pers.tile([P, DK, S], F16)
        WQ = pers.tile([P, CT, DK, P], F16)
        WK = pers.tile([P, CT, DK, P], F16)
        WV = pers.tile([P, CT, DK, P], F16)
        WO = pers.tile([P, CT, D], F16)
        QT = pers.tile([P, CT, S], F16)
        KT = pers.tile([P, CT, S], F16)
        V4 = pers.tile([P, SC, HPC, VW], F16)
        CTXQ = pers.tile([P, SC, HPC, HD], F16)
        # c-major ctx, c-chunks interleaved per q-chunk so one XBAR dma
        # transpose per q-half produces it: CTXT[p, qh, qc, cc, q] holds
        # ctx^T[cc*128+p, qh*1024+qc*128+q]
        CTXT = pers.tile([P, 2, 8, CT, P], F16)
        BQK = pers.tile([P, 2, CT], F32)

        etp = ctx.enter_context(tc.tile_pool(name="et", bufs=6))
        osbp = ctx.enter_context(tc.tile_pool(name="osb", bufs=16))
        recp = ctx.enter_context(tc.tile_pool(name="rec", bufs=4))
        stp = ctx.enter_context(tc.tile_pool(name="st", bufs=2, space="PSUM"))
        pop = ctx.enter_context(tc.tile_pool(name="po", bufs=1, space="PSUM"))
        jbp = ctx.enter_context(tc.tile_pool(name="jb", bufs=1, space="PSUM"))

        # V4 denominator-ones and pad columns (written once; V copies fill
        # the data columns)
        nc.gpsimd.memset(V4[:, :, :, HD], 1.0)
        nc.gpsimd.memset(V4[:, :, :, HD + 1], 0.0)
        WRM = pers.tile([1, 2], F16)
        nc.gpsimd.memset(WRM, 0.0)

        # ---- DMA: weights first as whole-tensor copies (HWDGE issue slots
        # are the scarce resource at ~650ns each), then s-progressive x^T
        # chunks so attention can start on the first q-half / k-chunks
        # while the rest streams in ----
        # wq/wk ct0 halves lead (the preamble's only weights, 364ns each
        # thanks to p-major packing), interleaved with the s[0:512] x^T
        # pairs the preamble consumes; everything else streams behind
        nc.sync.dma_start(WQ[:, 0], wq[:, 0])
        for dk in range(0, 4, 2):
            nc.sync.dma_start(XT[:, dk:dk + 2, 0:QW], xt_r[:, dk:dk + 2, 0:QW])
        nc.sync.dma_start(WK[:, 0], wk[:, 0])
        for dk in range(4, DK, 2):
            nc.sync.dma_start(XT[:, dk:dk + 2, 0:QW], xt_r[:, dk:dk + 2, 0:QW])
        nc.sync.dma_start(BQK, bqk.rearrange("qk (ct p) -> p qk ct", p=P))
        nc.sync.dma_start(WV[:, 0], wv[:, 0])
        nc.sync.dma_start(WV[:, 1], wv[:, 1])
        for dk in range(0, DK, 2):
            nc.sync.dma_start(XT[:, dk:dk + 2, QW:PW], xt_r[:, dk:dk + 2, QW:PW])
        nc.sync.dma_start(WQ[:, 1], wq[:, 1])
        nc.sync.dma_start(WK[:, 1], wk[:, 1])
        for dk in range(0, DK, 2):
            nc.sync.dma_start(XT[:, dk:dk + 2, PW:S], xt_r[:, dk:dk + 2, PW:S])
        nc.sync.dma_start(WO, wo[:, :, :])

        # warm-up: a negligible matmul as early as possible starts the PE
        # p-state ramp (~11us to full clock) during the DMA preamble
        wps = jbp.tile([1, 2], F32, tag="jb", name="wps")
        nc.tensor.matmul(wps, lhsT=WRM[:, 0:1], rhs=WRM, start=True, stop=True)

        # ---- preamble: Q ct0 q[0:512] plus K ct0 k-chunks 0-1, chunk-paced
        # against the x^T DMA stream (one Q + two K matmuls fit in a chunk
        # interval), then Q q[512:1024]. Pass 0 starts half-width so the
        # exp stream fires as soon as the first Q half is copied. ----
        qa = stp.tile([P, QW], F32, tag="st", name="preQa")
        kp = jbp.tile([P, 2 * P], F32, tag="jb", name="preK")
        for dk in range(DK):
            nc.tensor.matmul(
                qa, lhsT=WQ[:, 0, dk, :], rhs=XT[:, dk, 0:QW],
                start=(dk == 0), stop=(dk == DK - 1))
            nc.tensor.matmul(
                kp, lhsT=WK[:, 0, dk, :], rhs=XT[:, dk, 0:2 * P],
                start=(dk == 0), stop=(dk == DK - 1))
        nc.vector.tensor_scalar_add(QT[:, 0, 0:QW], qa, BQK[:, 0, 0:1])
        nc.vector.tensor_scalar_add(KT[:, 0, 0:2 * P], kp, BQK[:, 1, 0:1])

        def q_second_half():
            qb = stp.tile([P, QW], F32, tag="st", name="preQb")

            def mm(dk):
                return lambda: nc.tensor.matmul(
                    qb, lhsT=WQ[:, 0, dk, :], rhs=XT[:, dk, QW:PW],
                    start=(dk == 0), stop=(dk == DK - 1))

            def fin():
                nc.vector.tensor_scalar_add(
                    QT[:, 0, QW:PW], qb, BQK[:, 0, 0:1])
            return [mm(dk) for dk in range(DK)], fin

        # ---- deferred QKV work as jobs ----
        def v_job(st, vhp):
            """V for k-chunk st, head-pair vhp only (the pass that consumes
            a head-pair also computes its V, halving pass-0's job load)."""
            jb = jbp.tile([P, P], F32, tag="jb", name=f"vj{vhp}_{st}")

            def mm(dk):
                return lambda: nc.tensor.matmul(
                    jb, lhsT=XT[:, dk, st * P:(st + 1) * P],
                    rhs=WV[:, vhp, dk, :],
                    start=(dk == 0), stop=(dk == DK - 1))

            def fin():
                nc.vector.tensor_copy(
                    V4[:, st, 2 * vhp:2 * vhp + 2, 0:HD],
                    jb.rearrange("p (h d) -> p h d", d=HD))
            return [mm(dk) for dk in range(DK)], fin

        def k_chunk_job(ct_i, kc):
            jb = jbp.tile([P, P], F32, tag="jb", name=f"kj{ct_i}_{kc}")

            def mm(dk):
                return lambda: nc.tensor.matmul(
                    jb, lhsT=WK[:, ct_i, dk, :],
                    rhs=XT[:, dk, kc * P:(kc + 1) * P],
                    start=(dk == 0), stop=(dk == DK - 1))

            def fin():
                nc.vector.tensor_scalar_add(
                    KT[:, ct_i, kc * P:(kc + 1) * P], jb,
                    BQK[:, 1, ct_i:ct_i + 1])
            return [mm(dk) for dk in range(DK)], fin

        def q_job(ct_i, qc):
            jb = jbp.tile([P, QW], F32, tag="jb", name=f"qj{ct_i}_{qc}")

            def mm(dk):
                return lambda: nc.tensor.matmul(
                    jb, lhsT=WQ[:, ct_i, dk, :],
                    rhs=XT[:, dk, qc * QW:(qc + 1) * QW],
                    start=(dk == 0), stop=(dk == DK - 1))

            def fin():
                nc.vector.tensor_scalar_add(
                    QT[:, ct_i, qc * QW:(qc + 1) * QW], jb,
                    BQK[:, 0, ct_i:ct_i + 1])
            return [mm(dk) for dk in range(DK)], fin

        # per-pass job queues. Sub-iteration index it = kc*2 + hh (0..31).
        # Jobs are interleaved in deadline order so V chunks and K chunks
        # arrive just in time for the kc loop that consumes them.
        pass_jobs = [_Jobs() for _ in range(4)]
        p0 = []
        # pass 0 ((hp0,qh0)): Q q[512:1024] paced across the half-width
        # units, V st1.. JIT, K ct0 kc4..15 JIT, Q ct0 qh1
        # (V st0 is emitted inline right after the third pass-0 unit)
        # v_ready[hp]: k-chunks whose V copy has been emitted (attn@V for a
        # chunk must not be emitted before its V job — program order is the
        # only ordering guarantee)
        v_ready = [set(), set()]

        def v_job_r(st_i, vhp):
            mms, fin = v_job(st_i, vhp)

            def fin_r():
                fin()
                v_ready[vhp].add(st_i)
            return mms, fin_r

        # pass-0 unit index of the first score matmul that reads K chunk kc
        def p0_k_deadline(kc):
            return 2 * kc if kc < 4 else 16 + 2 * (kc - 4)

        for kc in (2, 3):
            mms, fin = k_chunk_job(0, kc)
            p0.append((kc - 2, mms, fin, P, p0_k_deadline(kc) - 1))
        mms, fin = q_second_half()
        p0.append((1, mms, fin, QW, 7))
        for st_i in range(1, SC):
            mms, fin = v_job_r(st_i, 0)
            p0.append((max(1, 2 * st_i + 2), mms, fin, P, None))
        for kc in range(4, SC):
            mms, fin = k_chunk_job(0, kc)
            p0.append((max(1, 2 * kc), mms, fin, P, p0_k_deadline(kc) - 1))
        for qc in (2, 3):
            mms, fin = q_job(0, qc)
            p0.append((14 + 4 * qc, mms, fin, QW, None))
        for due, mms, fin, rows, dl in sorted(p0, key=lambda t: t[0]):
            pass_jobs[0].add(mms, fin, due=due, rows=rows, deadline=dl)
        # pass 1 ((hp0,qh1)): K ct1 all chunks, Q ct1 qh0
        for kc in range(SC):
            mms, fin = k_chunk_job(1, kc)
            pass_jobs[1].add(mms, fin, due=2 * kc, rows=P)
        for qc in (0, 1):
            mms, fin = q_job(1, qc)
            pass_jobs[1].add(mms, fin, due=20 + 5 * qc, rows=QW)
        # pass 2 ((hp1,qh0)): V for heads 2-3 JIT, Q ct1 qh1
        for st_i in range(SC):
            mms, fin = v_job_r(st_i, 1)
            pass_jobs[2].add(mms, fin, due=max(0, 2 * st_i - 4), rows=P)
        for qc in (2, 3):
            mms, fin = q_job(1, qc)
            pass_jobs[2].add(mms, fin, due=8 * qc, rows=QW)

        # ---- out-projection helpers ----
        def oproj_unit(nn, qh, store_eng, slot=1):
            """out^T[nn*128:+128, qh*1024:+1024] = WO^T-chunk @ CTXT: four
            bank-aligned matmuls into one 2-bank psum tile, one wide store,
            one DMA (fewer stores/sems than per-512 units)."""
            pool, tag = ((jbp, "jb"), (stp, "st"), (pop, "po"))[slot]
            ps = pool.tile([P, PW], F32, tag=tag, name=f"op{nn}_{qh}")
            for half in range(2):
                for cc in range(CT):
                    nc.tensor.matmul(
                        ps[:, half * QW:(half + 1) * QW],
                        lhsT=WO[:, cc, nn * P:(nn + 1) * P],
                        rhs=CTXT[:, qh, half * 4:half * 4 + 4, cc, :],
                        start=(cc == 0), stop=(cc == CT - 1))
            osb = osbp.tile([P, PW], F16, tag="osb", name=f"osb{nn}_{qh}")
            # halves drain on both store engines concurrently
            nc.scalar.copy(osb[:, 0:QW], ps[:, 0:QW])
            nc.vector.tensor_copy(osb[:, QW:PW], ps[:, QW:PW])
            nc.sync.dma_start(
                out[nn * P:(nn + 1) * P, qh * PW:(qh + 1) * PW], osb)

        # out-proj for qh0 runs inside pass 3 ((hp1,qh1)) as per-512 halves
        # through the single jb bank (the score ring is busy), paced
        osb_q0 = {}

        def oproj_half(nn, half, slot=0):
            pool, tag = ((jbp, "jb"), (stp, "st"), (pop, "po"))[slot]
            ps = pool.tile([P, QW], F32, tag=tag, name=f"oph{nn}_{half}")
            for cc in range(CT):
                nc.tensor.matmul(
                    ps, lhsT=WO[:, cc, nn * P:(nn + 1) * P],
                    rhs=CTXT[:, 0, half * 4:half * 4 + 4, cc, :],
                    start=(cc == 0), stop=(cc == CT - 1))
            if nn not in osb_q0:
                osb_q0[nn] = osbp.tile([P, PW], F16, tag="osb",
                                       name=f"osbq0_{nn}")
            nc.vector.tensor_copy(
                osb_q0[nn][:, half * QW:(half + 1) * QW], ps)
            if half == 1:
                nc.sync.dma_start(
                    out[nn * P:(nn + 1) * P, 0:PW], osb_q0.pop(nn))

        op_halves = deque((nn, h) for nn in range(DK) for h in (0, 1))

        # ---- 4 attention passes of (head-pair, q-half) ----
        # Pass 0's first eight kc are split into half-width (512-q) units
        # so exps start right after the first Q half lands; V st0 and the
        # second Q half are slotted between those early units.
        for pi, (hp, qh) in enumerate(((0, 0), (0, 1), (1, 0), (1, 1))):
            po = pop.tile([P, 8, 2, VW], F32, tag="po", name=f"po{pi}")
            # 16 (m, hh) accumulation groups share po's banks; a start=True
            # would pending-zero a whole 2KB bank and wipe its siblings, so
            # zero the tile once and accumulate throughout
            nc.vector.memset(po, 0.0)
            q0 = qh * PW

            def attn_v(pend):
                et, kc, hh, mlo, mhi = pend
                for m in range(mlo, mhi):
                    nc.tensor.matmul(
                        po[:, m, hh, :],
                        lhsT=et[:, (m - mlo) * P:(m - mlo + 1) * P],
                        rhs=V4[:, kc, 2 * hp + hh, :],
                        start=False, stop=(kc == SC - 1),
                        skip_group_check=True)

            if pi == 0:
                units = ([(kc, hh, 0, 1) for kc in range(4) for hh in (0, 1)]
                         + [(kc, hh, 1, 2) for kc in range(4) for hh in (0, 1)]
                         + [(kc, hh, 0, 2) for kc in range(4, SC)
                            for hh in (0, 1)])
            else:
                units = [(kc, hh, 0, 2) for kc in range(SC) for hh in (0, 1)]

            pends = []
            for it, (kc, hh, jlo, jhi) in enumerate(units):
                pass_jobs[pi].force(it)
                w = (jhi - jlo) * QW
                st = stp.tile([P, w], F32, tag="st", name=f"st{pi}")
                for j in range(jlo, jhi):
                    nc.tensor.matmul(
                        st[:, (j - jlo) * QW:(j - jlo + 1) * QW],
                        lhsT=KT[hh * 64:(hh + 1) * 64, hp,
                                kc * P:(kc + 1) * P],
                        rhs=QT[hh * 64:(hh + 1) * 64, hp,
                               q0 + j * QW:q0 + (j + 1) * QW],
                        start=True, stop=True)
                while len(pends) >= 2 and pends[0][1] in v_ready[hp]:
                    attn_v(pends.pop(0))
                et = etp.tile([P, w], F16, tag="et", name="et")
                nc.scalar.activation(et, st, AF.Exp, scale=0.125)
                pends.append((et, kc, hh, jlo * 4, jhi * 4))
                if pi == 0 and it == 0:
                    # V chunk 0, needed by the first attn@V two units later
                    mms, fin = v_job_r(0, 0)
                    for m in mms:
                        m()
                    fin()
                else:
                    # pace the queue evenly over the pass's remaining units
                    rem = pass_jobs[pi].remaining_rows()
                    pace = (rem * 13) // (10 * max(1, len(units) - it))
                    pass_jobs[pi].emit(it, max(512, pace))
                if pi == 3 and op_halves and it % 2 == 1 and it >= 17:
                    oproj_half(*op_halves.popleft())
            pass_jobs[pi].drain()
            for pend in pends:
                attn_v(pend)

            # normalize: ctxq[q, h, hd] = po[q, h, hd] / po[q, h, 64];
            # split per q-half-of-pass on the last pass so the transpose
            # and the tail out-projection start as early as possible
            halves = ((0, 4), (4, 8)) if pi == 3 else ((0, 8),)
            for lo, hi in halves:
                rec = recp.tile([P, hi - lo, 2], F32, tag="rec", name="rec")
                nc.vector.reciprocal(rec, po[:, lo:hi, :, HD])
                nc.vector.tensor_mul(
                    CTXQ[:, qh * 8 + lo:qh * 8 + hi, 2 * hp:2 * hp + 2, :],
                    po[:, lo:hi, :, 0:HD],
                    rec.unsqueeze(3).to_broadcast((P, hi - lo, 2, HD)))
                if hp == 1:
                    # both head-pairs of this q-half done: XBAR transpose
                    # on the (idle) DMA engines makes the c-major form.
                    # The last pass issues from the ACT queue (idle after
                    # its final exp), skipping the SP-queue backlog on the
                    # critical tail path.
                    eng = nc.scalar if pi == 3 else nc.sync
                    eng.dma_start_transpose(
                        CTXT[:, qh, lo:hi].rearrange(
                            "p qc cc q -> p (qc cc) q"),
                        CTXQ[:, qh * 8 + lo:qh * 8 + hi])
                if pi == 3 and lo == 0:
                    # a few bridge rows between the two norm halves keep
                    # the PE hot and their stores ahead of norm-B on DVE
                    for _ in range(3):
                        if op_halves:
                            oproj_half(*op_halves.popleft())

        # leftover qh0 work bridges the transpose latency (it only needs
        # the long-ready qh0 ctx), keeping the PE hot for the qh1 tail;
        # everything streams as wide units over the three 2-bank psum
        # slots (st x2 + the freed po) with both store engines in play
        while op_halves:
            oproj_half(*op_halves.popleft())
        for n_tail, nn in enumerate(range(DK)):
            oproj_unit(nn, 1, "both", slot=1 + (n_tail % 3 == 2))

        if dbg:
            nc.sync.dma_start(d_qt[:, :], QT.rearrange("p a b -> p (a b)"))
            nc.sync.dma_start(d_kt[:, :], KT.rearrange("p a b -> p (a b)"))
            nc.sync.dma_start(d_v4[:, :], V4.rearrange("p a b c -> p (a b c)"))
            nc.sync.dma_start(d_cq[:, :], CTXQ.rearrange("p a b c -> p (a b c)"))
            nc.sync.dma_start(d_ct[:, :], CTXT.rearrange("p a b c d -> p (a b c d)"))

    nc.compile()
    return nc


_NC = None


def _pack_w(w):
    # [D, C] -> p-major [p, ct, dk, c0] so DMA rows are >=2KB
    return np.ascontiguousarray(
        w.reshape(DK, P, CT, P).transpose(1, 2, 0, 3)).astype(np.float16)


def core_inputs(core, x, w_qkv, b_qkv, w_out):
    b_i, g = divmod(core, CORES_PER_BATCH)
    cs = slice(g * HPC * HD, (g + 1) * HPC * HD)  # this core's channels
    return {
        "xt": np.ascontiguousarray(x[b_i].T).astype(np.float16),
        "wq": _pack_w(w_qkv[:, 0 * D:1 * D][:, cs]),
        "wk": _pack_w(w_qkv[:, 1 * D:2 * D][:, cs]),
        "wv": _pack_w(w_qkv[:, 2 * D:3 * D][:, cs]),
        "wo": np.ascontiguousarray(
            w_out[cs, :].reshape(CT, P, D).transpose(1, 0, 2)
        ).astype(np.float16),
        "bqk": np.ascontiguousarray(
            np.stack([b_qkv[0 * D:1 * D][cs], b_qkv[1 * D:2 * D][cs]])),
    }


def kernel(x, w_qkv, b_qkv, w_out, b_out):
    global _NC
    x = np.asarray(x, dtype=np.float32)
    w_qkv = np.asarray(w_qkv, dtype=np.float32)
    b_qkv = np.asarray(b_qkv, dtype=np.float32)
    w_out = np.asarray(w_out, dtype=np.float32)
    b_out = np.asarray(b_out, dtype=np.float32)

    if _NC is None:
        _NC = _build()

    in_maps = [core_inputs(core, x, w_qkv, b_qkv, w_out)
               for core in range(N_CORES)]

    trace = bool(int(os.environ.get("BASS_KERNEL_TRACE", "0")))
    res = run_bass_kernel_spmd(
        _NC, in_maps, core_ids=list(range(N_CORES)), trace=trace,
    )
    if trace and res.exec_time_ns is not None:
        print(f"HW exec time: {res.exec_time_ns} ns")
        if res.instructions_and_trace is not None:
            print(f"trace: {res.instructions_and_trace[1]}")

    # b_out and the V-bias term (softmax weights sum to 1, so x@w_v bias
    # b_v contributes the constant row b_v @ w_out) are host-folded.
    bias_row = b_out + b_qkv[2 * D:3 * D] @ w_out

    outs = [r["out"] for r in res.results]
    full = np.empty((B, S, D), dtype=np.float32)
    for b_i in range(B):
        acc = np.sum(
            np.stack(outs[b_i * CORES_PER_BATCH:(b_i + 1) * CORES_PER_BATCH]),
            axis=0, dtype=np.float32,
        ).T
        full[b_i] = acc + bias_row[None, :]
    return full
